# revision 6
# baseline (speedup 1.0000x reference)
"""Trainium2 Bass kernel for nn_CausalSelfAttention_17188459119385.

Sharding: 8 cores = batch (2) x KV-head groups (4).  Core c handles batch
c//4 and KV head c%4 (with its 4 grouped query heads).  Each core computes
a partial output y_part = attn_out @ w_o[rows of its heads]; the host sums
the 4 partials per batch and adds b_o.

v2 design (all matmul operands bf16, fp32 PSUM accumulation):
  - x is transposed on HOST: xt dram [C, T] bf16, DMA'd straight into the
    x^T SBUF layout the projections need (no PE transposes).
  - Q^T = wq^T x^T as before.  K and V are projected in ONE fused pass:
    stationary [wv | wk] -> KVt [128, T] with V^T on partitions 0:64 and
    K^T on 64:128.  K^T is DMA-copied to partitions 0:64 of a second tile
    (Kta) so the two row-tiles of the score pair have aligned stationaries.
    V natural [T, D] is recovered with 16 small PE transposes (64x128).
  - Scores for a head PAIR run as two concurrent 64-contraction matmuls on
    disjoint PE row-groups (tile_position (0,0)/(64,0)) sharing the same
    K block (GQA!), writing two PSUM banks of one [128, 2, 512] tile.
  - ONE strided exp per (pair, s-block) covers both heads' scores
    ([128, 2, 512-j0] AP) -> halves ACT instruction-overhead vs per-head.
  - PV per head with the ones-column rowsum trick; normalization via
    reciprocal + gpsimd partition_broadcast as before.
  - Emission interleaves next-ti projections between attention chains so
    the PE queue always has LDW-hideable independent work.
  - y output bf16 (halves the output DMA; host sums partials in fp32).
"""

import sys

if "/opt/trn_rl_repo" not in sys.path:
    sys.path.insert(0, "/opt/trn_rl_repo")

import numpy as np
import ml_dtypes

B, T, C = 2, 2048, 1024
NKV, G, D = 4, 4, 64          # kv heads, q-heads per kv head, head dim
QD = G * D                    # 256: q-feature width per core
P = 128
TCH = 512                     # t-chunk (matmul moving width)
NT = T // TCH                 # 4
NCC = C // P                  # 8 contraction chunks
NS = T // P                   # 16 s-blocks
BF16 = ml_dtypes.bfloat16

_CACHE = {}


def _build_nc():
    import concourse.mybir as mybir
    from concourse import bacc
    from concourse.tile import TileContext

    dt = mybir.dt
    AF = mybir.ActivationFunctionType

    nc = bacc.Bacc("TRN2", target_bir_lowering=False, debug=False)

    xtd = nc.dram_tensor("xtd", [C, T], dt.bfloat16, kind="ExternalInput")
    wq = nc.dram_tensor("wq", [C, QD], dt.bfloat16, kind="ExternalInput")
    wkv = nc.dram_tensor("wkv", [C, 2 * D], dt.bfloat16, kind="ExternalInput")
    wo = nc.dram_tensor("wo", [QD, C], dt.bfloat16, kind="ExternalInput")
    bq = nc.dram_tensor("bq", [P, 2], dt.float32, kind="ExternalInput")
    bkv = nc.dram_tensor("bkv", [P, 1], dt.float32, kind="ExternalInput")
    msk = nc.dram_tensor("msk", [P, P], dt.bfloat16, kind="ExternalInput")
    idin = nc.dram_tensor("idin", [P, P], dt.bfloat16, kind="ExternalInput")
    yt = nc.dram_tensor("yt", [C, T], dt.bfloat16, kind="ExternalOutput")

    with TileContext(nc) as tc:
        with (
            tc.tile_pool(name="const", bufs=1) as cpool,
            tc.tile_pool(name="xt", bufs=NCC) as xtpool,
            tc.tile_pool(name="qt", bufs=2) as qtpool,
            tc.tile_pool(name="kvt", bufs=1) as kvtpool,
            tc.tile_pool(name="kta", bufs=1) as ktapool,
            tc.tile_pool(name="v", bufs=1) as vpool,
            tc.tile_pool(name="ot", bufs=2) as otpool,
            tc.tile_pool(name="p", bufs=4) as ppool,
            tc.tile_pool(name="r", bufs=8) as rpool,
            tc.tile_pool(name="rbs", bufs=4) as rbspool,
            tc.tile_pool(name="y", bufs=4) as ypool,
            tc.tile_pool(name="otmp", bufs=4) as otmp,
            tc.tile_pool(name="mmps", bufs=2, space="PSUM") as mmps,
            tc.tile_pool(name="sps", bufs=2, space="PSUM") as sps,
            tc.tile_pool(name="ops", bufs=2, space="PSUM") as opspool,
        ):
            # ---- constants; DMA issue (~617ns/inst) spread over the three
            # DGE-capable queues (sync, scalar HWDGE; gpsimd SWDGE) --------
            # dummy exp first on scalar: preloads the exp table set (~2.7us)
            # while the input DMAs stream.
            scr = cpool.tile([1, 2], dt.float32, tag="scr")
            nc.vector.memset(scr[:, 0:1], 0.0)
            AFexp = AF.Exp
            nc.scalar.activation(scr[:, 1:2], scr[:, 0:1], AFexp)

            wq_sb = cpool.tile([P, NCC, QD], dt.bfloat16, tag="wq")
            nc.sync.dma_start(wq_sb[:], wq.ap().rearrange("(a p) d -> p a d", p=P))
            wkv_sb = cpool.tile([P, NCC, 2 * D], dt.bfloat16, tag="wkv")
            nc.sync.dma_start(wkv_sb[:], wkv.ap().rearrange("(a p) d -> p a d", p=P))
            msk_sb = cpool.tile([P, P], dt.bfloat16, tag="msk")
            nc.scalar.dma_start(msk_sb[:], msk[:])

            # x^T: ti-major so ti=0 lands first; a 0-3 on sync, 4-7 on
            # scalar for ti<2, gpsimd for ti>=2 (keeps scalar free for exp)
            xt = [xtpool.tile([P, T], dt.bfloat16, tag="xt", name=f"xt{a}")
                  for a in range(NCC)]
            for ti in range(NT):
                for a in range(NCC):
                    if a < 4:
                        eng = nc.sync
                    elif ti < 2:
                        eng = nc.scalar
                    else:
                        eng = nc.gpsimd
                    eng.dma_start(
                        xt[a][:, ti * TCH:(ti + 1) * TCH],
                        xtd[a * P:(a + 1) * P, ti * TCH:(ti + 1) * TCH])

            ident = cpool.tile([P, P], dt.bfloat16, tag="ident")
            nc.sync.dma_start(ident[:], idin[:])
            bq_sb = cpool.tile([P, 2], dt.float32, tag="bq")
            nc.sync.dma_start(bq_sb[:], bq[:])
            bkv_sb = cpool.tile([P, 1], dt.float32, tag="bkv")
            nc.sync.dma_start(bkv_sb[:], bkv[:])
            wo_sb = cpool.tile([P, 2, C], dt.bfloat16, tag="wo")
            nc.scalar.dma_start(wo_sb[:], wo.ap().rearrange("(a p) e -> p a e", p=P))

            # ---- persistent tensors ----
            Qt = [qtpool.tile([P, T], dt.bfloat16, tag="qt", name=f"qt{i}")
                  for i in range(2)]
            KVt = kvtpool.tile([P, T], dt.bfloat16, tag="kvt")
            Kta = ktapool.tile([D, T], dt.bfloat16, tag="kta")
            Vb = vpool.tile([P, NS, D + 1], dt.bfloat16, tag="v")
            nc.gpsimd.memset(Vb[:], 1.0)
            Ot = [otpool.tile([P, T], dt.bfloat16, tag="ot", name=f"ot{i}")
                  for i in range(2)]

            def proj_q(ti):
                t0 = ti * TCH
                for qc in range(2):
                    ps = mmps.tile([P, TCH], dt.float32, tag="mm")
                    for a in range(NCC):
                        nc.tensor.matmul(
                            ps[:], wq_sb[:, a, qc * P:(qc + 1) * P],
                            xt[a][:, t0:t0 + TCH],
                            start=(a == 0), stop=(a == NCC - 1))
                    nc.vector.tensor_scalar_add(
                        Qt[qc][:, t0:t0 + TCH], ps[:], bq_sb[:, qc:qc + 1])

            def proj_kv(ti):
                t0 = ti * TCH
                ps = mmps.tile([P, TCH], dt.float32, tag="mm")
                for a in range(NCC):
                    nc.tensor.matmul(
                        ps[:], wkv_sb[:, a, :], xt[a][:, t0:t0 + TCH],
                        start=(a == 0), stop=(a == NCC - 1))
                nc.vector.tensor_scalar_add(
                    KVt[:, t0:t0 + TCH], ps[:], bkv_sb[:, 0:1])
                # K^T dup to partitions 0:64 for the row-tile-A stationary
                nc.sync.dma_start(
                    Kta[:, t0:t0 + TCH], KVt[D:2 * D, t0:t0 + TCH])
                # V natural via 4 PE transposes of V^T 128-blocks
                for tt in range(4):
                    si = ti * 4 + tt
                    tp = mmps.tile([P, D], dt.bfloat16, tag="mm")
                    nc.tensor.transpose(
                        tp[:], KVt[0:D, si * P:(si + 1) * P],
                        ident[0:D, 0:D])
                    nc.vector.tensor_copy(Vb[:, si, 0:D], tp[:])

            def attn(ti, qc, filler=()):
                filler = list(filler)
                t0 = ti * TCH
                nsb = (t0 + TCH) // P
                o_psA = opspool.tile([D + 1, TCH], dt.float32, tag="o",
                                     name="opsA")
                o_psB = opspool.tile([D + 1, TCH], dt.float32, tag="o",
                                     name="opsB")
                pending = None
                for sb in range(nsb):
                    if filler:
                        filler.pop(0)()
                    s0 = sb * P
                    j0 = max(s0 - t0, 0)
                    sp = sps.tile([P, 2, TCH], dt.float32, tag="s")
                    nc.tensor.matmul(
                        sp[:, 0, j0:], Kta[:, s0:s0 + P],
                        Qt[qc][0:D, t0 + j0:t0 + TCH],
                        start=True, stop=True)
                    nc.tensor.matmul(
                        sp[:, 1, j0:], KVt[D:2 * D, s0:s0 + P],
                        Qt[qc][D:2 * D, t0 + j0:t0 + TCH],
                        start=True, stop=True)
                    pt = ppool.tile([P, 2, TCH], dt.bfloat16, tag="p")
                    nc.scalar.activation(pt[:, :, j0:], sp[:, :, j0:],
                                         AF.Exp, scale=0.125)
                    if s0 >= t0:
                        nc.vector.tensor_mul(
                            pt[:, 0, j0:j0 + P], pt[:, 0, j0:j0 + P],
                            msk_sb[:])
                        nc.vector.tensor_mul(
                            pt[:, 1, j0:j0 + P], pt[:, 1, j0:j0 + P],
                            msk_sb[:])
                    if pending is not None:
                        psb, pj0, ppt = pending
                        nc.tensor.matmul(o_psA[:, pj0:], Vb[:, psb, :],
                                         ppt[:, 0, pj0:],
                                         start=(psb == 0), stop=False)
                        nc.tensor.matmul(o_psB[:, pj0:], Vb[:, psb, :],
                                         ppt[:, 1, pj0:],
                                         start=(psb == 0), stop=False)
                    pending = (sb, j0, pt)
                psb, pj0, ppt = pending
                nc.tensor.matmul(o_psA[:, pj0:], Vb[:, psb, :],
                                 ppt[:, 0, pj0:],
                                 start=(psb == 0), stop=True)
                nc.tensor.matmul(o_psB[:, pj0:], Vb[:, psb, :],
                                 ppt[:, 1, pj0:],
                                 start=(psb == 0), stop=True)
                # normalize: row D of o_ps is the rowsum
                for h, o_ps in ((0, o_psA), (1, o_psB)):
                    rs = rpool.tile([1, TCH], dt.float32, tag="rs")
                    nc.vector.tensor_copy(rs[:], o_ps[D:D + 1, :])
                    rr = rpool.tile([1, TCH], dt.float32, tag="rr")
                    nc.vector.reciprocal_approx_fast(rr[:], rs[:])
                    rb_sb = rbspool.tile([D, TCH], dt.float32, tag="rbs")
                    nc.gpsimd.partition_broadcast(rb_sb[:], rr[:])
                    if h == 0:
                        nc.vector.tensor_mul(
                            Ot[qc][0:D, t0:t0 + TCH], o_ps[0:D, :], rb_sb[:])
                    else:
                        ott = otmp.tile([D, TCH], dt.bfloat16, tag="ott")
                        nc.vector.tensor_mul(ott[:], o_ps[0:D, :], rb_sb[:])
                        nc.sync.dma_start(
                            Ot[qc][D:2 * D, t0:t0 + TCH], ott[:])

            def proj_o_ec(ti, ec, on_scalar=False):
                t0 = ti * TCH

                def emit():
                    y_ps = mmps.tile([P, TCH], dt.float32, tag="mm")
                    for dc in range(2):
                        nc.tensor.matmul(
                            y_ps[:], wo_sb[:, dc, ec * P:(ec + 1) * P],
                            Ot[dc][:, t0:t0 + TCH],
                            start=(dc == 0), stop=(dc == 1))
                    y_sb = ypool.tile([P, TCH], dt.bfloat16, tag="y")
                    if on_scalar:
                        nc.scalar.copy(y_sb[:], y_ps[:])
                    else:
                        nc.vector.tensor_copy(y_sb[:], y_ps[:])
                    nc.sync.dma_start(
                        yt[ec * P:(ec + 1) * P, t0:t0 + TCH], y_sb[:])
                return emit

            # ---- schedule: proj_o of ti-1 is injected between the s-block
            # chains of ti's attention so the PE queue never starves while
            # ACT paces the softmax; the final proj_o runs on its own.
            proj_q(0)
            proj_kv(0)
            attn(0, 0)
            proj_q(1)
            attn(0, 1)
            proj_kv(1)
            for ti in range(1, NT):
                fa = [proj_o_ec(ti - 1, ec) for ec in range(8)]
                if ti < NT - 1:
                    attn(ti, 0, filler=fa)
                    proj_q(ti + 1)
                    attn(ti, 1)
                    proj_kv(ti + 1)
                else:
                    attn(ti, 0, filler=fa[:4])
                    attn(ti, 1, filler=fa[4:])
            for ec in range(8):
                proj_o_ec(NT - 1, ec, on_scalar=True)()

    nc.compile()
    return nc


def get_nc():
    if "nc" not in _CACHE:
        _CACHE["nc"] = _build_nc()
    return _CACHE["nc"]


def make_in_maps(x, w_q, b_q, w_k, b_k, w_v, b_v, w_o, b_o):
    """Host-side sharding: per-core input maps for cores 0..7."""
    tri = np.triu(np.ones((P, P), np.float32)).astype(BF16)  # keep s<=t
    in_maps = []
    for c in range(8):
        b, kv = divmod(c, NKV)
        q0 = kv * QD
        in_maps.append({
            "xtd": np.ascontiguousarray(x[b].T).astype(BF16),
            "wq": np.ascontiguousarray(w_q[:, q0:q0 + QD]).astype(BF16),
            "wkv": np.ascontiguousarray(np.concatenate(
                [w_v[:, kv * D:(kv + 1) * D],
                 w_k[:, kv * D:(kv + 1) * D]], axis=1)).astype(BF16),
            "wo": np.ascontiguousarray(w_o[q0:q0 + QD, :]).astype(BF16),
            "bq": np.ascontiguousarray(
                b_q[q0:q0 + QD].astype(np.float32).reshape(2, P).T),
            "bkv": np.concatenate(
                [b_v[kv * D:(kv + 1) * D], b_k[kv * D:(kv + 1) * D]]
            ).astype(np.float32).reshape(P, 1),
            "msk": tri,
            "idin": np.eye(P, dtype=np.float32).astype(BF16),
        })
    return in_maps


def kernel(x, w_q, b_q, w_k, b_k, w_v, b_v, w_o, b_o):
    from concourse.bass_utils import run_bass_kernel_spmd

    x = np.asarray(x)
    nc = get_nc()
    in_maps = make_in_maps(x, np.asarray(w_q), np.asarray(b_q),
                           np.asarray(w_k), np.asarray(b_k),
                           np.asarray(w_v), np.asarray(b_v),
                           np.asarray(w_o), np.asarray(b_o))
    res = run_bass_kernel_spmd(nc, in_maps, list(range(8)))
    out = np.zeros((B, T, C), np.float32)
    for c in range(8):
        out[c // NKV] += res.results[c]["yt"].astype(np.float32).T
    out += np.asarray(b_o).astype(np.float32)[None, None, :]
    return out


# revision 10
# speedup vs baseline: 1.0337x; 1.0337x over previous
"""Trainium2 Bass kernel for nn_CausalSelfAttention_17188459119385.

Sharding: 8 cores = batch (2) x KV-head groups (4).  Core c handles batch
c//4 and KV head c%4 (with its 4 grouped query heads).  Each core computes
a partial output y_part = attn_out @ w_o[rows of its heads]; the host sums
the 4 partials per batch and adds b_o.

v2 design (all matmul operands bf16, fp32 PSUM accumulation):
  - x is transposed on HOST: xt dram [C, T] bf16, DMA'd straight into the
    x^T SBUF layout the projections need (no PE transposes).
  - Q^T = wq^T x^T as before.  K and V are projected in ONE fused pass:
    stationary [wv | wk] -> KVt [128, T] with V^T on partitions 0:64 and
    K^T on 64:128.  K^T is DMA-copied to partitions 0:64 of a second tile
    (Kta) so the two row-tiles of the score pair have aligned stationaries.
    V natural [T, D] is recovered with 16 small PE transposes (64x128).
  - Scores for a head PAIR run as two concurrent 64-contraction matmuls on
    disjoint PE row-groups (tile_position (0,0)/(64,0)) sharing the same
    K block (GQA!), writing two PSUM banks of one [128, 2, 512] tile.
  - ONE strided exp per (pair, s-block) covers both heads' scores
    ([128, 2, 512-j0] AP) -> halves ACT instruction-overhead vs per-head.
  - PV per head with the ones-column rowsum trick; normalization via
    reciprocal + gpsimd partition_broadcast as before.
  - Emission interleaves next-ti projections between attention chains so
    the PE queue always has LDW-hideable independent work.
  - y output bf16 (halves the output DMA; host sums partials in fp32).
"""

import sys

if "/opt/trn_rl_repo" not in sys.path:
    sys.path.insert(0, "/opt/trn_rl_repo")

import numpy as np
import ml_dtypes

B, T, C = 2, 2048, 1024
NKV, G, D = 4, 4, 64          # kv heads, q-heads per kv head, head dim
QD = G * D                    # 256: q-feature width per core
P = 128
TCH = 512                     # t-chunk (matmul moving width)
NT = T // TCH                 # 4
NCC = C // P                  # 8 contraction chunks
NS = T // P                   # 16 s-blocks
BF16 = ml_dtypes.bfloat16

_CACHE = {}


def _build_nc():
    import concourse.mybir as mybir
    from concourse import bacc
    from concourse.tile import TileContext

    dt = mybir.dt
    AF = mybir.ActivationFunctionType

    nc = bacc.Bacc("TRN2", target_bir_lowering=False, debug=False)

    xtd = nc.dram_tensor("xtd", [C, T], dt.bfloat16, kind="ExternalInput")
    wq = nc.dram_tensor("wq", [C, QD], dt.bfloat16, kind="ExternalInput")
    wkv = nc.dram_tensor("wkv", [C, 2 * D], dt.bfloat16, kind="ExternalInput")
    wo = nc.dram_tensor("wo", [QD, C], dt.bfloat16, kind="ExternalInput")
    bq = nc.dram_tensor("bq", [P, 2], dt.float32, kind="ExternalInput")
    bkv = nc.dram_tensor("bkv", [P, 1], dt.float32, kind="ExternalInput")
    msk = nc.dram_tensor("msk", [P, P], dt.bfloat16, kind="ExternalInput")
    idin = nc.dram_tensor("idin", [P, P], dt.bfloat16, kind="ExternalInput")
    yt = nc.dram_tensor("yt", [C, T], dt.bfloat16, kind="ExternalOutput")

    with TileContext(nc) as tc:
        with (
            tc.tile_pool(name="const", bufs=1) as cpool,
            tc.tile_pool(name="xt", bufs=NCC) as xtpool,
            tc.tile_pool(name="qt", bufs=2) as qtpool,
            tc.tile_pool(name="kvt", bufs=1) as kvtpool,
            tc.tile_pool(name="kta", bufs=1) as ktapool,
            tc.tile_pool(name="v", bufs=1) as vpool,
            tc.tile_pool(name="ot", bufs=2) as otpool,
            tc.tile_pool(name="p", bufs=4) as ppool,
            tc.tile_pool(name="r", bufs=8) as rpool,
            tc.tile_pool(name="rbs", bufs=4) as rbspool,
            tc.tile_pool(name="y", bufs=4) as ypool,
            tc.tile_pool(name="otmp", bufs=4) as otmp,
            tc.tile_pool(name="mmps", bufs=2, space="PSUM") as mmps,
            tc.tile_pool(name="sps", bufs=2, space="PSUM") as sps,
            tc.tile_pool(name="ops", bufs=2, space="PSUM") as opspool,
        ):
            # ---- constants; DMA issue (~617ns/inst) spread over the three
            # DGE-capable queues (sync, scalar HWDGE; gpsimd SWDGE) --------
            # dummy exp first on scalar: preloads the exp table set (~2.7us)
            # while the input DMAs stream.
            scr = cpool.tile([1, 2], dt.float32, tag="scr")
            nc.vector.memset(scr[:, 0:1], 0.0)
            AFexp = AF.Exp
            nc.scalar.activation(scr[:, 1:2], scr[:, 0:1], AFexp)

            wq_sb = cpool.tile([P, NCC, QD], dt.bfloat16, tag="wq")
            nc.sync.dma_start(wq_sb[:], wq.ap().rearrange("(a p) d -> p a d", p=P))
            wkv_sb = cpool.tile([P, NCC, 2 * D], dt.bfloat16, tag="wkv")
            nc.sync.dma_start(wkv_sb[:], wkv.ap().rearrange("(a p) d -> p a d", p=P))
            msk_sb = cpool.tile([P, P], dt.bfloat16, tag="msk")
            nc.scalar.dma_start(msk_sb[:], msk[:])

            # x^T: ti-major on sync so ti=0 lands first (one queue keeps
            # the critical-set transfers front-of-line; scalar carries only
            # what attention needs later)
            xt = [xtpool.tile([P, T], dt.bfloat16, tag="xt", name=f"xt{a}")
                  for a in range(NCC)]
            ident = cpool.tile([P, P], dt.bfloat16, tag="ident")
            bq_sb = cpool.tile([P, 2], dt.float32, tag="bq")
            bkv_sb = cpool.tile([P, 1], dt.float32, tag="bkv")
            for ti in range(NT):
                for a in range(NCC):
                    nc.sync.dma_start(
                        xt[a][:, ti * TCH:(ti + 1) * TCH],
                        xtd[a * P:(a + 1) * P, ti * TCH:(ti + 1) * TCH])
                if ti == 0:
                    nc.sync.dma_start(ident[:], idin[:])
                    nc.sync.dma_start(bq_sb[:], bq[:])
                    nc.sync.dma_start(bkv_sb[:], bkv[:])

            wo_sb = cpool.tile([P, 2, C], dt.bfloat16, tag="wo")
            nc.scalar.dma_start(wo_sb[:], wo.ap().rearrange("(a p) e -> p a e", p=P))

            # ---- persistent tensors ----
            Qt = [qtpool.tile([P, T], dt.bfloat16, tag="qt", name=f"qt{i}")
                  for i in range(2)]
            KVt = kvtpool.tile([P, T], dt.bfloat16, tag="kvt")
            Kta = ktapool.tile([D, T], dt.bfloat16, tag="kta")
            Vb = vpool.tile([P, NS, D + 1], dt.bfloat16, tag="v")
            nc.gpsimd.memset(Vb[:], 1.0)
            Ot = [otpool.tile([P, T], dt.bfloat16, tag="ot", name=f"ot{i}")
                  for i in range(2)]

            def proj_q_qc(ti, qc):
                def emit():
                    t0 = ti * TCH
                    ps = mmps.tile([P, TCH], dt.float32, tag="mm")
                    for a in range(NCC):
                        nc.tensor.matmul(
                            ps[:], wq_sb[:, a, qc * P:(qc + 1) * P],
                            xt[a][:, t0:t0 + TCH],
                            start=(a == 0), stop=(a == NCC - 1))
                    nc.vector.tensor_scalar_add(
                        Qt[qc][:, t0:t0 + TCH], ps[:], bq_sb[:, qc:qc + 1])
                return emit

            def proj_q(ti):
                proj_q_qc(ti, 0)()
                proj_q_qc(ti, 1)()

            def proj_kv(ti):
                t0 = ti * TCH
                ps = mmps.tile([P, TCH], dt.float32, tag="mm")
                for a in range(NCC):
                    nc.tensor.matmul(
                        ps[:], wkv_sb[:, a, :], xt[a][:, t0:t0 + TCH],
                        start=(a == 0), stop=(a == NCC - 1))
                nc.vector.tensor_scalar_add(
                    KVt[:, t0:t0 + TCH], ps[:], bkv_sb[:, 0:1])
                # K^T dup to partitions 0:64 for the row-tile-A stationary
                nc.sync.dma_start(
                    Kta[:, t0:t0 + TCH], KVt[D:2 * D, t0:t0 + TCH])
                # V natural via 4 PE transposes of V^T 128-blocks
                for tt in range(4):
                    si = ti * 4 + tt
                    tp = mmps.tile([P, D], dt.bfloat16, tag="mm")
                    nc.tensor.transpose(
                        tp[:], KVt[0:D, si * P:(si + 1) * P],
                        ident[0:D, 0:D])
                    nc.vector.tensor_copy(Vb[:, si, 0:D], tp[:])

            def attn(ti, qc, filler=()):
                filler = list(filler)
                t0 = ti * TCH
                nsb = (t0 + TCH) // P
                o_psA = opspool.tile([D + 1, TCH], dt.float32, tag="o",
                                     name="opsA")
                o_psB = opspool.tile([D + 1, TCH], dt.float32, tag="o",
                                     name="opsB")
                pending = None
                for sb in range(nsb):
                    # consume fillers in the LAST len(filler) slots: that is
                    # where ACT pacing starves the PE
                    if filler and sb >= nsb - len(filler):
                        filler.pop(0)()
                    s0 = sb * P
                    j0 = max(s0 - t0, 0)
                    sp = sps.tile([P, 2, TCH], dt.float32, tag="s")
                    nc.tensor.matmul(
                        sp[:, 0, j0:], Kta[:, s0:s0 + P],
                        Qt[qc][0:D, t0 + j0:t0 + TCH],
                        start=True, stop=True)
                    nc.tensor.matmul(
                        sp[:, 1, j0:], KVt[D:2 * D, s0:s0 + P],
                        Qt[qc][D:2 * D, t0 + j0:t0 + TCH],
                        start=True, stop=True)
                    pt = ppool.tile([P, 2, TCH], dt.bfloat16, tag="p")
                    nc.scalar.activation(pt[:, :, j0:], sp[:, :, j0:],
                                         AF.Exp, scale=0.125)
                    if s0 >= t0:
                        nc.vector.tensor_mul(
                            pt[:, 0, j0:j0 + P], pt[:, 0, j0:j0 + P],
                            msk_sb[:])
                        nc.vector.tensor_mul(
                            pt[:, 1, j0:j0 + P], pt[:, 1, j0:j0 + P],
                            msk_sb[:])
                    if pending is not None:
                        psb, pj0, ppt = pending
                        nc.tensor.matmul(o_psA[:, pj0:], Vb[:, psb, :],
                                         ppt[:, 0, pj0:],
                                         start=(psb == 0), stop=False)
                        nc.tensor.matmul(o_psB[:, pj0:], Vb[:, psb, :],
                                         ppt[:, 1, pj0:],
                                         start=(psb == 0), stop=False)
                    pending = (sb, j0, pt)
                psb, pj0, ppt = pending
                nc.tensor.matmul(o_psA[:, pj0:], Vb[:, psb, :],
                                 ppt[:, 0, pj0:],
                                 start=(psb == 0), stop=True)
                nc.tensor.matmul(o_psB[:, pj0:], Vb[:, psb, :],
                                 ppt[:, 1, pj0:],
                                 start=(psb == 0), stop=True)
                # normalize: row D of o_ps is the rowsum
                for h, o_ps in ((0, o_psA), (1, o_psB)):
                    rs = rpool.tile([1, TCH], dt.float32, tag="rs")
                    nc.vector.tensor_copy(rs[:], o_ps[D:D + 1, :])
                    rr = rpool.tile([1, TCH], dt.float32, tag="rr")
                    nc.vector.reciprocal_approx_fast(rr[:], rs[:])
                    rb_sb = rbspool.tile([D, TCH], dt.float32, tag="rbs")
                    nc.gpsimd.partition_broadcast(rb_sb[:], rr[:])
                    if h == 0:
                        nc.vector.tensor_mul(
                            Ot[qc][0:D, t0:t0 + TCH], o_ps[0:D, :], rb_sb[:])
                    else:
                        ott = otmp.tile([D, TCH], dt.bfloat16, tag="ott")
                        nc.vector.tensor_mul(ott[:], o_ps[0:D, :], rb_sb[:])
                        nc.sync.dma_start(
                            Ot[qc][D:2 * D, t0:t0 + TCH], ott[:])

            def proj_o_ec(ti, ec, on_scalar=False):
                t0 = ti * TCH

                def emit():
                    y_ps = mmps.tile([P, TCH], dt.float32, tag="mm")
                    for dc in range(2):
                        nc.tensor.matmul(
                            y_ps[:], wo_sb[:, dc, ec * P:(ec + 1) * P],
                            Ot[dc][:, t0:t0 + TCH],
                            start=(dc == 0), stop=(dc == 1))
                    y_sb = ypool.tile([P, TCH], dt.bfloat16, tag="y")
                    if on_scalar:
                        nc.scalar.copy(y_sb[:], y_ps[:])
                    else:
                        nc.vector.tensor_copy(y_sb[:], y_ps[:])
                    nc.sync.dma_start(
                        yt[ec * P:(ec + 1) * P, t0:t0 + TCH], y_sb[:])
                return emit

            # ---- schedule: independent matmul work (proj_o of ti-1, the
            # projections of ti+1) is injected into the tail slots of each
            # attention chain, where ACT pacing would otherwise starve the
            # PE queue.
            proj_q(0)
            proj_kv(0)
            attn(0, 0)
            attn(0, 1, filler=[proj_q_qc(1, 0), proj_q_qc(1, 1),
                               lambda: proj_kv(1)])
            for ti in range(1, NT):
                fa = [proj_o_ec(ti - 1, ec) for ec in range(8)]
                if ti < NT - 1:
                    attn(ti, 0, filler=fa)
                    attn(ti, 1, filler=[proj_q_qc(ti + 1, 0),
                                        proj_q_qc(ti + 1, 1),
                                        (lambda t=ti: proj_kv(t + 1))])
                else:
                    attn(ti, 0, filler=fa[:4])
                    attn(ti, 1, filler=fa[4:])
            for ec in range(8):
                proj_o_ec(NT - 1, ec, on_scalar=True)()

    nc.compile()
    return nc


def get_nc():
    if "nc" not in _CACHE:
        _CACHE["nc"] = _build_nc()
    return _CACHE["nc"]


def make_in_maps(x, w_q, b_q, w_k, b_k, w_v, b_v, w_o, b_o):
    """Host-side sharding: per-core input maps for cores 0..7."""
    tri = np.triu(np.ones((P, P), np.float32)).astype(BF16)  # keep s<=t
    in_maps = []
    for c in range(8):
        b, kv = divmod(c, NKV)
        q0 = kv * QD
        in_maps.append({
            "xtd": np.ascontiguousarray(x[b].T).astype(BF16),
            "wq": np.ascontiguousarray(w_q[:, q0:q0 + QD]).astype(BF16),
            "wkv": np.ascontiguousarray(np.concatenate(
                [w_v[:, kv * D:(kv + 1) * D],
                 w_k[:, kv * D:(kv + 1) * D]], axis=1)).astype(BF16),
            "wo": np.ascontiguousarray(w_o[q0:q0 + QD, :]).astype(BF16),
            "bq": np.ascontiguousarray(
                b_q[q0:q0 + QD].astype(np.float32).reshape(2, P).T),
            "bkv": np.concatenate(
                [b_v[kv * D:(kv + 1) * D], b_k[kv * D:(kv + 1) * D]]
            ).astype(np.float32).reshape(P, 1),
            "msk": tri,
            "idin": np.eye(P, dtype=np.float32).astype(BF16),
        })
    return in_maps


def kernel(x, w_q, b_q, w_k, b_k, w_v, b_v, w_o, b_o):
    from concourse.bass_utils import run_bass_kernel_spmd

    x = np.asarray(x)
    nc = get_nc()
    in_maps = make_in_maps(x, np.asarray(w_q), np.asarray(b_q),
                           np.asarray(w_k), np.asarray(b_k),
                           np.asarray(w_v), np.asarray(b_v),
                           np.asarray(w_o), np.asarray(b_o))
    res = run_bass_kernel_spmd(nc, in_maps, list(range(8)))
    out = np.zeros((B, T, C), np.float32)
    for c in range(8):
        out[c // NKV] += res.results[c]["yt"].astype(np.float32).T
    out += np.asarray(b_o).astype(np.float32)[None, None, :]
    return out


# revision 12
# speedup vs baseline: 1.0682x; 1.0334x over previous
"""Trainium2 Bass kernel for nn_CausalSelfAttention_17188459119385.

Sharding: 8 cores = batch (2) x KV-head groups (4).  Core c handles batch
c//4 and KV head c%4 (with its 4 grouped query heads).  Each core computes
a partial output y_part = attn_out @ w_o[rows of its heads]; the host sums
the 4 partials per batch and adds b_o.

v2 design (all matmul operands bf16, fp32 PSUM accumulation):
  - x is transposed on HOST: xt dram [C, T] bf16, DMA'd straight into the
    x^T SBUF layout the projections need (no PE transposes).
  - Q^T = wq^T x^T as before.  K and V are projected in ONE fused pass:
    stationary [wv | wk] -> KVt [128, T] with V^T on partitions 0:64 and
    K^T on 64:128.  K^T is DMA-copied to partitions 0:64 of a second tile
    (Kta) so the two row-tiles of the score pair have aligned stationaries.
    V natural [T, D] is recovered with 16 small PE transposes (64x128).
  - Scores for a head PAIR run as two concurrent 64-contraction matmuls on
    disjoint PE row-groups (tile_position (0,0)/(64,0)) sharing the same
    K block (GQA!), writing two PSUM banks of one [128, 2, 512] tile.
  - ONE strided exp per (pair, s-block) covers both heads' scores
    ([128, 2, 512-j0] AP) -> halves ACT instruction-overhead vs per-head.
  - PV per head with the ones-column rowsum trick; normalization via
    reciprocal + gpsimd partition_broadcast as before.
  - Emission interleaves next-ti projections between attention chains so
    the PE queue always has LDW-hideable independent work.
  - y output bf16 (halves the output DMA; host sums partials in fp32).
"""

import sys

if "/opt/trn_rl_repo" not in sys.path:
    sys.path.insert(0, "/opt/trn_rl_repo")

import numpy as np
import ml_dtypes

B, T, C = 2, 2048, 1024
NKV, G, D = 4, 4, 64          # kv heads, q-heads per kv head, head dim
QD = G * D                    # 256: q-feature width per core
P = 128
TCH = 512                     # t-chunk (matmul moving width)
NT = T // TCH                 # 4
NCC = C // P                  # 8 contraction chunks
NS = T // P                   # 16 s-blocks
BF16 = ml_dtypes.bfloat16

_CACHE = {}


def _build_nc():
    import concourse.mybir as mybir
    from concourse import bacc
    from concourse.tile import TileContext

    dt = mybir.dt
    AF = mybir.ActivationFunctionType

    nc = bacc.Bacc("TRN2", target_bir_lowering=False, debug=False)

    xtd = nc.dram_tensor("xtd", [C, T], dt.bfloat16, kind="ExternalInput")
    wq = nc.dram_tensor("wq", [C, QD], dt.bfloat16, kind="ExternalInput")
    wkv = nc.dram_tensor("wkv", [C, 2 * D], dt.bfloat16, kind="ExternalInput")
    wo = nc.dram_tensor("wo", [QD, C], dt.bfloat16, kind="ExternalInput")
    bq = nc.dram_tensor("bq", [P, 2], dt.float32, kind="ExternalInput")
    bkv = nc.dram_tensor("bkv", [P, 1], dt.float32, kind="ExternalInput")
    msk = nc.dram_tensor("msk", [P, P], dt.bfloat16, kind="ExternalInput")
    idin = nc.dram_tensor("idin", [P, P], dt.bfloat16, kind="ExternalInput")
    yt = nc.dram_tensor("yt", [C, T], dt.bfloat16, kind="ExternalOutput")

    with TileContext(nc) as tc:
        with (
            tc.tile_pool(name="const", bufs=1) as cpool,
            tc.tile_pool(name="xt", bufs=NCC) as xtpool,
            tc.tile_pool(name="qt", bufs=2) as qtpool,
            tc.tile_pool(name="kvt", bufs=1) as kvtpool,
            tc.tile_pool(name="kta", bufs=1) as ktapool,
            tc.tile_pool(name="v", bufs=1) as vpool,
            tc.tile_pool(name="ot", bufs=2) as otpool,
            tc.tile_pool(name="p", bufs=4) as ppool,
            tc.tile_pool(name="r", bufs=8) as rpool,
            tc.tile_pool(name="rbs", bufs=4) as rbspool,
            tc.tile_pool(name="y", bufs=4) as ypool,
            tc.tile_pool(name="otmp", bufs=4) as otmp,
            tc.tile_pool(name="mmps", bufs=2, space="PSUM") as mmps,
            tc.tile_pool(name="sps", bufs=2, space="PSUM") as sps,
            tc.tile_pool(name="ops", bufs=2, space="PSUM") as opspool,
        ):
            # ---- constants; DMA issue (~617ns/inst) spread over the three
            # DGE-capable queues (sync, scalar HWDGE; gpsimd SWDGE) --------
            # dummy exp first on scalar: preloads the exp table set (~2.7us)
            # while the input DMAs stream.
            scr = cpool.tile([1, 2], dt.float32, tag="scr")
            nc.vector.memset(scr[:, 0:1], 0.0)
            AFexp = AF.Exp
            nc.scalar.activation(scr[:, 1:2], scr[:, 0:1], AFexp)

            wq_sb = cpool.tile([P, NCC, QD], dt.bfloat16, tag="wq")
            nc.sync.dma_start(wq_sb[:], wq.ap().rearrange("(a p) d -> p a d", p=P))
            wkv_sb = cpool.tile([P, NCC, 2 * D], dt.bfloat16, tag="wkv")
            nc.sync.dma_start(wkv_sb[:], wkv.ap().rearrange("(a p) d -> p a d", p=P))
            msk_sb = cpool.tile([P, P], dt.bfloat16, tag="msk")
            nc.scalar.dma_start(msk_sb[:], msk[:])

            # x^T: ti-major on sync so ti=0 lands first (one queue keeps
            # the critical-set transfers front-of-line; scalar carries only
            # what attention needs later)
            xt = [xtpool.tile([P, T], dt.bfloat16, tag="xt", name=f"xt{a}")
                  for a in range(NCC)]
            ident = cpool.tile([P, P], dt.bfloat16, tag="ident")
            bq_sb = cpool.tile([P, 2], dt.float32, tag="bq")
            bkv_sb = cpool.tile([P, 1], dt.float32, tag="bkv")
            def emit_xt(ti):
                for a in range(NCC):
                    nc.sync.dma_start(
                        xt[a][:, ti * TCH:(ti + 1) * TCH],
                        xtd[a * P:(a + 1) * P, ti * TCH:(ti + 1) * TCH])

            # only ti=0 up front: consumers wait on the whole queue emitted
            # so far, so later slices are emitted between attention chains
            emit_xt(0)
            nc.sync.dma_start(ident[:], idin[:])
            nc.sync.dma_start(bq_sb[:], bq[:])
            nc.sync.dma_start(bkv_sb[:], bkv[:])

            wo_sb = cpool.tile([P, 2, C], dt.bfloat16, tag="wo")
            nc.scalar.dma_start(wo_sb[:], wo.ap().rearrange("(a p) e -> p a e", p=P))

            # ---- persistent tensors ----
            Qt = [qtpool.tile([P, T], dt.bfloat16, tag="qt", name=f"qt{i}")
                  for i in range(2)]
            KVt = kvtpool.tile([P, T], dt.bfloat16, tag="kvt")
            Kta = ktapool.tile([D, T], dt.bfloat16, tag="kta")
            Vb = vpool.tile([P, NS, D + 1], dt.bfloat16, tag="v")
            nc.gpsimd.memset(Vb[:], 1.0)
            Ot = [otpool.tile([P, T], dt.bfloat16, tag="ot", name=f"ot{i}")
                  for i in range(2)]

            def proj_q_qc(ti, qc):
                def emit():
                    t0 = ti * TCH
                    ps = mmps.tile([P, TCH], dt.float32, tag="mm")
                    for a in range(NCC):
                        nc.tensor.matmul(
                            ps[:], wq_sb[:, a, qc * P:(qc + 1) * P],
                            xt[a][:, t0:t0 + TCH],
                            start=(a == 0), stop=(a == NCC - 1))
                    nc.vector.tensor_scalar_add(
                        Qt[qc][:, t0:t0 + TCH], ps[:], bq_sb[:, qc:qc + 1])
                return emit

            def proj_q(ti):
                proj_q_qc(ti, 0)()
                proj_q_qc(ti, 1)()

            def proj_kv(ti):
                t0 = ti * TCH
                ps = mmps.tile([P, TCH], dt.float32, tag="mm")
                for a in range(NCC):
                    nc.tensor.matmul(
                        ps[:], wkv_sb[:, a, :], xt[a][:, t0:t0 + TCH],
                        start=(a == 0), stop=(a == NCC - 1))
                nc.vector.tensor_scalar_add(
                    KVt[:, t0:t0 + TCH], ps[:], bkv_sb[:, 0:1])
                # K^T dup to partitions 0:64 for the row-tile-A stationary
                nc.sync.dma_start(
                    Kta[:, t0:t0 + TCH], KVt[D:2 * D, t0:t0 + TCH])
                # V natural via 4 PE transposes of V^T 128-blocks
                for tt in range(4):
                    si = ti * 4 + tt
                    tp = mmps.tile([P, D], dt.bfloat16, tag="mm")
                    nc.tensor.transpose(
                        tp[:], KVt[0:D, si * P:(si + 1) * P],
                        ident[0:D, 0:D])
                    nc.vector.tensor_copy(Vb[:, si, 0:D], tp[:])

            def attn(ti, qc, filler=()):
                filler = list(filler)
                t0 = ti * TCH
                nsb = (t0 + TCH) // P
                o_psA = opspool.tile([D + 1, TCH], dt.float32, tag="o",
                                     name="opsA")
                o_psB = opspool.tile([D + 1, TCH], dt.float32, tag="o",
                                     name="opsB")
                pending = None
                for sb in range(nsb):
                    # consume fillers in the LAST len(filler) slots: that is
                    # where ACT pacing starves the PE
                    if filler and sb >= nsb - len(filler):
                        filler.pop(0)()
                    s0 = sb * P
                    j0 = max(s0 - t0, 0)
                    sp = sps.tile([P, 2, TCH], dt.float32, tag="s")
                    nc.tensor.matmul(
                        sp[:, 0, j0:], Kta[:, s0:s0 + P],
                        Qt[qc][0:D, t0 + j0:t0 + TCH],
                        start=True, stop=True)
                    nc.tensor.matmul(
                        sp[:, 1, j0:], KVt[D:2 * D, s0:s0 + P],
                        Qt[qc][D:2 * D, t0 + j0:t0 + TCH],
                        start=True, stop=True)
                    pt = ppool.tile([P, 2, TCH], dt.bfloat16, tag="p")
                    nc.scalar.activation(pt[:, :, j0:], sp[:, :, j0:],
                                         AF.Exp, scale=0.125)
                    if s0 >= t0:
                        nc.vector.tensor_mul(
                            pt[:, 0, j0:j0 + P], pt[:, 0, j0:j0 + P],
                            msk_sb[:])
                        nc.vector.tensor_mul(
                            pt[:, 1, j0:j0 + P], pt[:, 1, j0:j0 + P],
                            msk_sb[:])
                    if pending is not None:
                        psb, pj0, ppt = pending
                        nc.tensor.matmul(o_psA[:, pj0:], Vb[:, psb, :],
                                         ppt[:, 0, pj0:],
                                         start=(psb == 0), stop=False)
                        nc.tensor.matmul(o_psB[:, pj0:], Vb[:, psb, :],
                                         ppt[:, 1, pj0:],
                                         start=(psb == 0), stop=False)
                    pending = (sb, j0, pt)
                psb, pj0, ppt = pending
                nc.tensor.matmul(o_psA[:, pj0:], Vb[:, psb, :],
                                 ppt[:, 0, pj0:],
                                 start=(psb == 0), stop=True)
                nc.tensor.matmul(o_psB[:, pj0:], Vb[:, psb, :],
                                 ppt[:, 1, pj0:],
                                 start=(psb == 0), stop=True)
                # normalize: row D of o_ps is the rowsum
                for h, o_ps in ((0, o_psA), (1, o_psB)):
                    rs = rpool.tile([1, TCH], dt.float32, tag="rs")
                    nc.vector.tensor_copy(rs[:], o_ps[D:D + 1, :])
                    rr = rpool.tile([1, TCH], dt.float32, tag="rr")
                    nc.vector.reciprocal_approx_fast(rr[:], rs[:])
                    rb_sb = rbspool.tile([D, TCH], dt.float32, tag="rbs")
                    nc.gpsimd.partition_broadcast(rb_sb[:], rr[:])
                    if h == 0:
                        nc.vector.tensor_mul(
                            Ot[qc][0:D, t0:t0 + TCH], o_ps[0:D, :], rb_sb[:])
                    else:
                        ott = otmp.tile([D, TCH], dt.bfloat16, tag="ott")
                        nc.vector.tensor_mul(ott[:], o_ps[0:D, :], rb_sb[:])
                        nc.sync.dma_start(
                            Ot[qc][D:2 * D, t0:t0 + TCH], ott[:])

            def proj_o_ec(ti, ec, on_scalar=False):
                t0 = ti * TCH

                def emit():
                    y_ps = mmps.tile([P, TCH], dt.float32, tag="mm")
                    for dc in range(2):
                        nc.tensor.matmul(
                            y_ps[:], wo_sb[:, dc, ec * P:(ec + 1) * P],
                            Ot[dc][:, t0:t0 + TCH],
                            start=(dc == 0), stop=(dc == 1))
                    y_sb = ypool.tile([P, TCH], dt.bfloat16, tag="y")
                    if on_scalar:
                        nc.scalar.copy(y_sb[:], y_ps[:])
                    else:
                        nc.vector.tensor_copy(y_sb[:], y_ps[:])
                    nc.sync.dma_start(
                        yt[ec * P:(ec + 1) * P, t0:t0 + TCH], y_sb[:])
                return emit

            # ---- schedule: independent matmul work (proj_o of ti-1, the
            # projections of ti+1) is injected into the tail slots of each
            # attention chain, where ACT pacing would otherwise starve the
            # PE queue.
            proj_q(0)
            proj_kv(0)
            emit_xt(1)
            attn(0, 0)
            attn(0, 1, filler=[proj_q_qc(1, 0), proj_q_qc(1, 1),
                               lambda: proj_kv(1)])
            for ti in range(1, NT):
                if ti + 1 < NT:
                    emit_xt(ti + 1)
                fa = [proj_o_ec(ti - 1, ec) for ec in range(8)]
                if ti < NT - 1:
                    attn(ti, 0, filler=fa)
                    attn(ti, 1, filler=[proj_q_qc(ti + 1, 0),
                                        proj_q_qc(ti + 1, 1),
                                        (lambda t=ti: proj_kv(t + 1))])
                else:
                    attn(ti, 0, filler=fa[:4])
                    attn(ti, 1, filler=fa[4:])
            for ec in range(8):
                proj_o_ec(NT - 1, ec, on_scalar=True)()

    nc.compile()
    return nc


def get_nc():
    if "nc" not in _CACHE:
        _CACHE["nc"] = _build_nc()
    return _CACHE["nc"]


def make_in_maps(x, w_q, b_q, w_k, b_k, w_v, b_v, w_o, b_o):
    """Host-side sharding: per-core input maps for cores 0..7."""
    tri = np.triu(np.ones((P, P), np.float32)).astype(BF16)  # keep s<=t
    in_maps = []
    for c in range(8):
        b, kv = divmod(c, NKV)
        q0 = kv * QD
        in_maps.append({
            "xtd": np.ascontiguousarray(x[b].T).astype(BF16),
            "wq": np.ascontiguousarray(w_q[:, q0:q0 + QD]).astype(BF16),
            "wkv": np.ascontiguousarray(np.concatenate(
                [w_v[:, kv * D:(kv + 1) * D],
                 w_k[:, kv * D:(kv + 1) * D]], axis=1)).astype(BF16),
            "wo": np.ascontiguousarray(w_o[q0:q0 + QD, :]).astype(BF16),
            "bq": np.ascontiguousarray(
                b_q[q0:q0 + QD].astype(np.float32).reshape(2, P).T),
            "bkv": np.concatenate(
                [b_v[kv * D:(kv + 1) * D], b_k[kv * D:(kv + 1) * D]]
            ).astype(np.float32).reshape(P, 1),
            "msk": tri,
            "idin": np.eye(P, dtype=np.float32).astype(BF16),
        })
    return in_maps


def kernel(x, w_q, b_q, w_k, b_k, w_v, b_v, w_o, b_o):
    from concourse.bass_utils import run_bass_kernel_spmd

    x = np.asarray(x)
    nc = get_nc()
    in_maps = make_in_maps(x, np.asarray(w_q), np.asarray(b_q),
                           np.asarray(w_k), np.asarray(b_k),
                           np.asarray(w_v), np.asarray(b_v),
                           np.asarray(w_o), np.asarray(b_o))
    res = run_bass_kernel_spmd(nc, in_maps, list(range(8)))
    out = np.zeros((B, T, C), np.float32)
    for c in range(8):
        out[c // NKV] += res.results[c]["yt"].astype(np.float32).T
    out += np.asarray(b_o).astype(np.float32)[None, None, :]
    return out


# revision 15
# speedup vs baseline: 1.0810x; 1.0120x over previous
"""Trainium2 Bass kernel for nn_CausalSelfAttention_17188459119385.

Sharding: 8 cores = batch (2) x KV-head groups (4).  Core c handles batch
c//4 and KV head c%4 (with its 4 grouped query heads).  Each core computes
a partial output y_part = attn_out @ w_o[rows of its heads]; the host sums
the 4 partials per batch and adds b_o.

v2 design (all matmul operands bf16, fp32 PSUM accumulation):
  - x is transposed on HOST: xt dram [C, T] bf16, DMA'd straight into the
    x^T SBUF layout the projections need (no PE transposes).
  - Q^T = wq^T x^T as before.  K and V are projected in ONE fused pass:
    stationary [wv | wk] -> KVt [128, T] with V^T on partitions 0:64 and
    K^T on 64:128.  K^T is DMA-copied to partitions 0:64 of a second tile
    (Kta) so the two row-tiles of the score pair have aligned stationaries.
    V natural [T, D] is recovered with 16 small PE transposes (64x128).
  - Scores for a head PAIR run as two concurrent 64-contraction matmuls on
    disjoint PE row-groups (tile_position (0,0)/(64,0)) sharing the same
    K block (GQA!), writing two PSUM banks of one [128, 2, 512] tile.
  - ONE strided exp per (pair, s-block) covers both heads' scores
    ([128, 2, 512-j0] AP) -> halves ACT instruction-overhead vs per-head.
  - PV per head with the ones-column rowsum trick; normalization via
    reciprocal + gpsimd partition_broadcast as before.
  - Emission interleaves next-ti projections between attention chains so
    the PE queue always has LDW-hideable independent work.
  - y output bf16 (halves the output DMA; host sums partials in fp32).
"""

import sys

if "/opt/trn_rl_repo" not in sys.path:
    sys.path.insert(0, "/opt/trn_rl_repo")

import numpy as np
import ml_dtypes

B, T, C = 2, 2048, 1024
NKV, G, D = 4, 4, 64          # kv heads, q-heads per kv head, head dim
QD = G * D                    # 256: q-feature width per core
P = 128
TCH = 512                     # t-chunk (matmul moving width)
NT = T // TCH                 # 4
NCC = C // P                  # 8 contraction chunks
NS = T // P                   # 16 s-blocks
BF16 = ml_dtypes.bfloat16

_CACHE = {}


def _build_nc():
    import concourse.mybir as mybir
    from concourse import bacc
    from concourse.tile import TileContext

    dt = mybir.dt
    AF = mybir.ActivationFunctionType

    nc = bacc.Bacc("TRN2", target_bir_lowering=False, debug=False)

    xtd = nc.dram_tensor("xtd", [C, T], dt.bfloat16, kind="ExternalInput")
    wq = nc.dram_tensor("wq", [C, QD], dt.bfloat16, kind="ExternalInput")
    wkv = nc.dram_tensor("wkv", [C, 2 * D], dt.bfloat16, kind="ExternalInput")
    wo = nc.dram_tensor("wo", [QD, C], dt.bfloat16, kind="ExternalInput")
    bq = nc.dram_tensor("bq", [P, 2], dt.float32, kind="ExternalInput")
    bkv = nc.dram_tensor("bkv", [P, 1], dt.float32, kind="ExternalInput")
    msk = nc.dram_tensor("msk", [P, P], dt.bfloat16, kind="ExternalInput")
    idin = nc.dram_tensor("idin", [P, P], dt.bfloat16, kind="ExternalInput")
    yt = nc.dram_tensor("yt", [C, T], dt.bfloat16, kind="ExternalOutput")

    with TileContext(nc) as tc:
        with (
            tc.tile_pool(name="const", bufs=1) as cpool,
            tc.tile_pool(name="xt", bufs=NCC) as xtpool,
            tc.tile_pool(name="qt", bufs=2) as qtpool,
            tc.tile_pool(name="kvt", bufs=1) as kvtpool,
            tc.tile_pool(name="kta", bufs=1) as ktapool,
            tc.tile_pool(name="v", bufs=1) as vpool,
            tc.tile_pool(name="ot", bufs=2) as otpool,
            tc.tile_pool(name="p", bufs=4) as ppool,
            tc.tile_pool(name="r", bufs=8) as rpool,
            tc.tile_pool(name="rbs", bufs=4) as rbspool,
            tc.tile_pool(name="y", bufs=4) as ypool,
            tc.tile_pool(name="otmp", bufs=4) as otmp,
            tc.tile_pool(name="mmps", bufs=2, space="PSUM") as mmps,
            tc.tile_pool(name="sps", bufs=2, space="PSUM") as sps,
            tc.tile_pool(name="ops", bufs=2, space="PSUM") as opspool,
        ):
            # ---- constants; DMA issue (~617ns/inst) spread over the three
            # DGE-capable queues (sync, scalar HWDGE; gpsimd SWDGE) --------
            # dummy exp first on scalar: preloads the exp table set (~2.7us)
            # while the input DMAs stream.
            scr = cpool.tile([1, 2], dt.float32, tag="scr")
            nc.vector.memset(scr[:, 0:1], 0.0)
            AFexp = AF.Exp
            nc.scalar.activation(scr[:, 1:2], scr[:, 0:1], AFexp)

            wq_sb = cpool.tile([P, NCC, QD], dt.bfloat16, tag="wq")
            nc.sync.dma_start(wq_sb[:], wq.ap().rearrange("(a p) d -> p a d", p=P))
            wkv_sb = cpool.tile([P, NCC, 2 * D], dt.bfloat16, tag="wkv")
            nc.sync.dma_start(wkv_sb[:], wkv.ap().rearrange("(a p) d -> p a d", p=P))
            msk_sb = cpool.tile([P, P], dt.bfloat16, tag="msk")
            nc.scalar.dma_start(msk_sb[:], msk[:])

            # x^T: ti-major on sync so ti=0 lands first (one queue keeps
            # the critical-set transfers front-of-line; scalar carries only
            # what attention needs later)
            xt = [xtpool.tile([P, T], dt.bfloat16, tag="xt", name=f"xt{a}")
                  for a in range(NCC)]
            ident = cpool.tile([P, P], dt.bfloat16, tag="ident")
            bq_sb = cpool.tile([P, 2], dt.float32, tag="bq")
            bkv_sb = cpool.tile([P, 1], dt.float32, tag="bkv")
            def emit_xt(ti):
                for a in range(NCC):
                    nc.sync.dma_start(
                        xt[a][:, ti * TCH:(ti + 1) * TCH],
                        xtd[a * P:(a + 1) * P, ti * TCH:(ti + 1) * TCH])

            # only ti=0 up front: consumers wait on the whole queue emitted
            # so far, so later slices are emitted between attention chains
            emit_xt(0)
            nc.sync.dma_start(ident[:], idin[:])
            nc.sync.dma_start(bq_sb[:], bq[:])
            nc.sync.dma_start(bkv_sb[:], bkv[:])

            wo_sb = cpool.tile([P, 2, C], dt.bfloat16, tag="wo")
            nc.scalar.dma_start(wo_sb[:], wo.ap().rearrange("(a p) e -> p a e", p=P))

            # ---- persistent tensors ----
            Qt = [qtpool.tile([P, T], dt.bfloat16, tag="qt", name=f"qt{i}")
                  for i in range(2)]
            KVt = kvtpool.tile([P, T], dt.bfloat16, tag="kvt")
            Kta = ktapool.tile([D, T], dt.bfloat16, tag="kta")
            Vb = vpool.tile([P, NS, D + 1], dt.bfloat16, tag="v")
            nc.gpsimd.memset(Vb[:], 1.0)
            Ot = [otpool.tile([P, T], dt.bfloat16, tag="ot", name=f"ot{i}")
                  for i in range(2)]

            def proj_q_qc(ti, qc):
                def emit():
                    t0 = ti * TCH
                    ps = mmps.tile([P, TCH], dt.float32, tag="mm")
                    for a in range(NCC):
                        nc.tensor.matmul(
                            ps[:], wq_sb[:, a, qc * P:(qc + 1) * P],
                            xt[a][:, t0:t0 + TCH],
                            start=(a == 0), stop=(a == NCC - 1))
                    nc.vector.tensor_scalar_add(
                        Qt[qc][:, t0:t0 + TCH], ps[:], bq_sb[:, qc:qc + 1])
                return emit

            def proj_q(ti):
                proj_q_qc(ti, 0)()
                proj_q_qc(ti, 1)()

            def proj_kv(ti):
                t0 = ti * TCH
                ps = mmps.tile([P, TCH], dt.float32, tag="mm")
                for a in range(NCC):
                    nc.tensor.matmul(
                        ps[:], wkv_sb[:, a, :], xt[a][:, t0:t0 + TCH],
                        start=(a == 0), stop=(a == NCC - 1))
                nc.vector.tensor_scalar_add(
                    KVt[:, t0:t0 + TCH], ps[:], bkv_sb[:, 0:1])
                # K^T dup to partitions 0:64 for the row-tile-A stationary
                nc.sync.dma_start(
                    Kta[:, t0:t0 + TCH], KVt[D:2 * D, t0:t0 + TCH])
                # V natural via 4 PE transposes of V^T 128-blocks
                for tt in range(4):
                    si = ti * 4 + tt
                    tp = mmps.tile([P, D], dt.bfloat16, tag="mm")
                    nc.tensor.transpose(
                        tp[:], KVt[0:D, si * P:(si + 1) * P],
                        ident[0:D, 0:D])
                    nc.vector.tensor_copy(Vb[:, si, 0:D], tp[:])

            def attn(ti, qc, filler=(), fshift=0):
                filler = list(filler)
                t0 = ti * TCH
                nsb = (t0 + TCH) // P
                o_psA = opspool.tile([D + 1, TCH], dt.float32, tag="o",
                                     name="opsA")
                o_psB = opspool.tile([D + 1, TCH], dt.float32, tag="o",
                                     name="opsB")
                pending = None
                for sb in range(nsb):
                    # one filler at the start (covers the o_ps handoff from
                    # the previous chain), the rest in the tail slots where
                    # ACT pacing starves the PE; fshift moves the tail batch
                    # clear of the diagonal mask run
                    if filler and (sb == 0 or
                                   sb >= nsb - len(filler) - fshift):
                        filler.pop(0)()
                    s0 = sb * P
                    j0 = max(s0 - t0, 0)
                    sp = sps.tile([P, 2, TCH], dt.float32, tag="s")
                    nc.tensor.matmul(
                        sp[:, 0, j0:], Kta[:, s0:s0 + P],
                        Qt[qc][0:D, t0 + j0:t0 + TCH],
                        start=True, stop=True)
                    nc.tensor.matmul(
                        sp[:, 1, j0:], KVt[D:2 * D, s0:s0 + P],
                        Qt[qc][D:2 * D, t0 + j0:t0 + TCH],
                        start=True, stop=True)
                    pt = ppool.tile([P, 2, TCH], dt.bfloat16, tag="p")
                    nc.scalar.activation(pt[:, :, j0:], sp[:, :, j0:],
                                         AF.Exp, scale=0.125)
                    if s0 >= t0:
                        nc.vector.tensor_mul(
                            pt[:, 0, j0:j0 + P], pt[:, 0, j0:j0 + P],
                            msk_sb[:])
                        nc.vector.tensor_mul(
                            pt[:, 1, j0:j0 + P], pt[:, 1, j0:j0 + P],
                            msk_sb[:])
                    if pending is not None:
                        psb, pj0, ppt = pending
                        nc.tensor.matmul(o_psA[:, pj0:], Vb[:, psb, :],
                                         ppt[:, 0, pj0:],
                                         start=(psb == 0), stop=False)
                        nc.tensor.matmul(o_psB[:, pj0:], Vb[:, psb, :],
                                         ppt[:, 1, pj0:],
                                         start=(psb == 0), stop=False)
                    pending = (sb, j0, pt)
                psb, pj0, ppt = pending
                nc.tensor.matmul(o_psA[:, pj0:], Vb[:, psb, :],
                                 ppt[:, 0, pj0:],
                                 start=(psb == 0), stop=True)
                nc.tensor.matmul(o_psB[:, pj0:], Vb[:, psb, :],
                                 ppt[:, 1, pj0:],
                                 start=(psb == 0), stop=True)
                # normalize: row D of o_ps is the rowsum
                for h, o_ps in ((0, o_psA), (1, o_psB)):
                    rs = rpool.tile([1, TCH], dt.float32, tag="rs")
                    nc.vector.tensor_copy(rs[:], o_ps[D:D + 1, :])
                    rr = rpool.tile([1, TCH], dt.float32, tag="rr")
                    nc.vector.reciprocal_approx_fast(rr[:], rs[:])
                    rb_sb = rbspool.tile([D, TCH], dt.float32, tag="rbs")
                    nc.gpsimd.partition_broadcast(rb_sb[:], rr[:])
                    if h == 0:
                        nc.vector.tensor_mul(
                            Ot[qc][0:D, t0:t0 + TCH], o_ps[0:D, :], rb_sb[:])
                    else:
                        ott = otmp.tile([D, TCH], dt.bfloat16, tag="ott")
                        nc.vector.tensor_mul(ott[:], o_ps[0:D, :], rb_sb[:])
                        nc.sync.dma_start(
                            Ot[qc][D:2 * D, t0:t0 + TCH], ott[:])

            def proj_o_ec(ti, ec, tail=False):
                t0 = ti * TCH

                def emit():
                    y_ps = mmps.tile([P, TCH], dt.float32, tag="mm")
                    for dc in range(2):
                        nc.tensor.matmul(
                            y_ps[:], wo_sb[:, dc, ec * P:(ec + 1) * P],
                            Ot[dc][:, t0:t0 + TCH],
                            start=(dc == 0), stop=(dc == 1))
                    y_sb = ypool.tile([P, TCH], dt.bfloat16, tag="y")
                    # at the kernel tail, fan copies/DMA issue across both
                    # engines so the drain pipeline is ~2x wider
                    if tail and ec % 2:
                        nc.scalar.copy(y_sb[:], y_ps[:])
                        nc.scalar.dma_start(
                            yt[ec * P:(ec + 1) * P, t0:t0 + TCH], y_sb[:])
                    else:
                        nc.vector.tensor_copy(y_sb[:], y_ps[:])
                        nc.sync.dma_start(
                            yt[ec * P:(ec + 1) * P, t0:t0 + TCH], y_sb[:])
                return emit

            # ---- schedule: independent matmul work (proj_o of ti-1, the
            # projections of ti+1) is injected into the tail slots of each
            # attention chain, where ACT pacing would otherwise starve the
            # PE queue.
            proj_q(0)
            proj_kv(0)
            emit_xt(1)
            attn(0, 0)
            attn(0, 1, filler=[proj_q_qc(1, 0), proj_q_qc(1, 1),
                               lambda: proj_kv(1)])
            for ti in range(1, NT):
                if ti + 1 < NT:
                    emit_xt(ti + 1)
                fa = [proj_o_ec(ti - 1, ec) for ec in range(8)]
                if ti < NT - 1:
                    attn(ti, 0, filler=fa)
                    attn(ti, 1, filler=[proj_q_qc(ti + 1, 0),
                                        proj_q_qc(ti + 1, 1),
                                        (lambda t=ti: proj_kv(t + 1))])
                else:
                    attn(ti, 0, filler=fa[:3], fshift=4)
                    attn(ti, 1, filler=fa[3:], fshift=4)
            for ec in range(8):
                proj_o_ec(NT - 1, ec, tail=True)()

    nc.compile()
    return nc


def get_nc():
    if "nc" not in _CACHE:
        _CACHE["nc"] = _build_nc()
    return _CACHE["nc"]


def make_in_maps(x, w_q, b_q, w_k, b_k, w_v, b_v, w_o, b_o):
    """Host-side sharding: per-core input maps for cores 0..7."""
    tri = np.triu(np.ones((P, P), np.float32)).astype(BF16)  # keep s<=t
    in_maps = []
    for c in range(8):
        b, kv = divmod(c, NKV)
        q0 = kv * QD
        in_maps.append({
            "xtd": np.ascontiguousarray(x[b].T).astype(BF16),
            "wq": np.ascontiguousarray(w_q[:, q0:q0 + QD]).astype(BF16),
            "wkv": np.ascontiguousarray(np.concatenate(
                [w_v[:, kv * D:(kv + 1) * D],
                 w_k[:, kv * D:(kv + 1) * D]], axis=1)).astype(BF16),
            "wo": np.ascontiguousarray(w_o[q0:q0 + QD, :]).astype(BF16),
            "bq": np.ascontiguousarray(
                b_q[q0:q0 + QD].astype(np.float32).reshape(2, P).T),
            "bkv": np.concatenate(
                [b_v[kv * D:(kv + 1) * D], b_k[kv * D:(kv + 1) * D]]
            ).astype(np.float32).reshape(P, 1),
            "msk": tri,
            "idin": np.eye(P, dtype=np.float32).astype(BF16),
        })
    return in_maps


def kernel(x, w_q, b_q, w_k, b_k, w_v, b_v, w_o, b_o):
    from concourse.bass_utils import run_bass_kernel_spmd

    x = np.asarray(x)
    nc = get_nc()
    in_maps = make_in_maps(x, np.asarray(w_q), np.asarray(b_q),
                           np.asarray(w_k), np.asarray(b_k),
                           np.asarray(w_v), np.asarray(b_v),
                           np.asarray(w_o), np.asarray(b_o))
    res = run_bass_kernel_spmd(nc, in_maps, list(range(8)))
    out = np.zeros((B, T, C), np.float32)
    for c in range(8):
        out[c // NKV] += res.results[c]["yt"].astype(np.float32).T
    out += np.asarray(b_o).astype(np.float32)[None, None, :]
    return out
